# revision 1
# baseline (speedup 1.0000x reference)
"""DMPNN encoder on 8 trn2 NeuronCores (Bass/Tile).

Formulation (exploits linearity of the neighbor-sum):
    inp   = f_bonds @ W_i                       (per-bond, row major)
    msg_0 = relu(inp)
    u_t   = msg_t @ W (W_h for t=0,1; W_o2 for t=2)   [per-bond shard, local]
    (allgather U_t across cores)
    msg_{t+1} = relu(inp + sum_k U_t[a2b[b2a[j],k]] - U_t[b2revb[j]])
    readout: a_msg = sum_k Z[a2b[a,k]];  hT = relu(W_o1.T@f_atomsT + W_o2.T@a_msgT + b_o)
    out = mean-pool hT over 20-atom molecules (transposed), host re-transposes.

Sharding: bonds and atoms split contiguously across 8 cores. All gathers are
indirect DMAs from allgathered (Shared DRAM) tables of premultiplied rows, so
the random a2b/b2a/b2revb indexing never crosses a core's local HBM.
"""
import sys
import types
import numpy as np
import ml_dtypes
from contextlib import ExitStack
from dataclasses import dataclass

import concourse.bass as bass
import concourse.mybir as mybir
import concourse.tile as tile
from concourse.masks import make_identity
from concourse.bass_utils import run_bass_kernel_spmd

P = 128
BF16 = mybir.dt.bfloat16
F32 = mybir.dt.float32
I32 = mybir.dt.int32
AO = mybir.AluOpType


# ---------------------------------------------------------------- wait split
_seq = [0]


def _mk_event(engine, waits, debug):
    _seq[0] += 1
    return mybir.InstEventSemaphore(
        name=f"WS-{_seq[0]}", engine=engine, ins=[], outs=[], debug=debug,
        sync_info=mybir.SyncInfo(on_wait=list(waits), on_update=[]),
    )


def split_waits(nc, cap_normal=1, cap_event=2):
    """This container's walrus caps sync waits at 1/inst (2 for
    EventSemaphore); Tile emits more. Move excess waits onto preceding
    EventSemaphore instructions on the same engine."""
    n_split = 0
    for fn in nc.m.functions:
        for blk in fn.blocks:
            insts = list(blk.instructions)
            out = []
            for inst in insts:
                si = inst.sync_info
                waits = list(si.on_wait) if (si is not None and si.on_wait) else []
                cap = cap_event if isinstance(inst, mybir.InstEventSemaphore) else cap_normal
                if len(waits) > cap:
                    extra, keep = waits[:-cap], waits[-cap:]
                    for i in range(0, len(extra), cap_event):
                        out.append(_mk_event(inst.engine, extra[i:i + cap_event], inst.debug))
                    inst.sync_info = mybir.SyncInfo(on_wait=keep, on_update=list(si.on_update or []))
                    n_split += 1
                out.append(inst)
            if len(out) != len(insts):
                blk.instructions = out
    return n_split


def install_ntff_shim():
    if "antenv.axon_hooks" in sys.modules:
        return
    mod = types.ModuleType("antenv.axon_hooks")
    state = {"hook": None}
    mod.set_axon_ntff_profile_hook = lambda h: state.__setitem__("hook", h)
    mod.get_axon_ntff_profile_hook = lambda: state["hook"]
    sys.modules["antenv.axon_hooks"] = mod
    import antenv
    antenv.axon_hooks = mod
    try:
        from trn_agent_boot.trn_boot import _ntff_profile_via_ctypes
        mod.set_axon_ntff_profile_hook(_ntff_profile_via_ctypes("/opt/axon/libaxon_pjrt.so"))
    except Exception:
        pass


# ---------------------------------------------------------------- geometry
@dataclass
class Geo:
    n_atoms: int
    n_bonds: int
    hidden: int
    atom_fd: int
    bond_fd: int
    n_cores: int
    apm: int  # atoms per mol

    def __post_init__(g):
        assert g.n_bonds % g.n_cores == 0 and g.n_atoms % g.n_cores == 0
        g.BS = g.n_bonds // g.n_cores
        g.AS = g.n_atoms // g.n_cores
        g.BT = -(-g.BS // P)            # bond tiles per core
        g.BSP = g.BT * P
        g.mols_per_core = g.AS // g.apm
        # atom group = lcm(P, apm) atoms so molecule boundaries align
        import math
        g.AG = math.lcm(P, g.apm)       # atoms per pool group
        g.TPG = g.AG // P               # tiles per group
        g.MPG = g.AG // g.apm           # mols per group
        g.NG = -(-g.AS // g.AG)         # groups per core
        g.AT = g.NG * g.TPG             # atom tiles per core (padded)
        g.ASP = g.AT * P
        # hidden chunks of <=128
        g.KC = []
        o = 0
        while o < g.hidden:
            g.KC.append((o, min(P, g.hidden - o)))
            o += P
        # bond_fd chunks
        g.FC = []
        o = 0
        while o < g.bond_fd:
            g.FC.append((o, min(P, g.bond_fd - o)))
            o += P
        # atom_fd chunks
        g.AC = []
        o = 0
        while o < g.atom_fd:
            g.AC.append((o, min(P, g.atom_fd - o)))
            o += P


DEFAULT_GEO = dict(n_atoms=100_000, n_bonds=200_000, hidden=300, atom_fd=133,
                   bond_fd=147, n_cores=8, apm=20)


# ---------------------------------------------------------------- device program
def build(g: Geo, depth: int = 3, debug: bool = False):
    H = g.hidden
    nc = bass.Bass(num_devices=g.n_cores)
    amsg_dbg = nc.dram_tensor("amsg_dbg", [g.ASP, H], BF16) if debug else None
    ht_dbg = nc.dram_tensor("ht_dbg", [len(g.KC), P, g.ASP], F32) if debug else None

    fbT = nc.declare_dram_parameter("fbT", [g.bond_fd, g.BSP], BF16, isOutput=False)
    faT = nc.declare_dram_parameter("faT", [g.atom_fd, g.ASP], BF16, isOutput=False)
    gidx = nc.declare_dram_parameter("gidx", [P, g.BT * 7], I32, isOutput=False)
    ridx = nc.declare_dram_parameter("ridx", [P, g.AT * 6], I32, isOutput=False)
    Wi = nc.declare_dram_parameter("Wi", [g.bond_fd, H], BF16, isOutput=False)
    Wh = nc.declare_dram_parameter("Wh", [H, H], BF16, isOutput=False)
    Wo1 = nc.declare_dram_parameter("Wo1", [g.atom_fd, H], BF16, isOutput=False)
    Wo2 = nc.declare_dram_parameter("Wo2", [H, H], BF16, isOutput=False)
    bo = nc.declare_dram_parameter("bo", [P, len(g.KC)], F32, isOutput=False)
    outT = nc.declare_dram_parameter("outT", [len(g.KC), P, g.NG * g.MPG], F32, isOutput=True)

    n_up = depth - 1  # message update rounds (gather passes)
    # local premultiplied tables + allgathered Shared versions
    Ul = [nc.dram_tensor(f"U{t}l", [g.BSP, H], BF16) for t in range(n_up + 1)]
    Uf = [nc.dram_tensor(f"U{t}f", [g.n_cores * g.BSP, H], BF16, addr_space="Shared")
          for t in range(n_up + 1)]

    def ag_halves(t):
        nc.gpsimd.collective_compute(
            "AllGather", AO.bypass, replica_groups=rg,
            ins=[Ul[t][:, :].opt()], outs=[Uf[t][:, :].opt()])

    rg = [list(range(g.n_cores))]

    with tile.TileContext(nc) as tc, ExitStack() as ctx:
        cst = ctx.enter_context(tc.tile_pool(name="cst", bufs=1))
        inp_p = ctx.enter_context(tc.tile_pool(name="inp", bufs=1))
        fbp = ctx.enter_context(tc.tile_pool(name="fbp", bufs=2))
        gp = ctx.enter_context(tc.tile_pool(name="gp", bufs=4))
        msgp = ctx.enter_context(tc.tile_pool(name="msgp", bufs=3))
        ltp = ctx.enter_context(tc.tile_pool(name="ltp", bufs=3))
        up = ctx.enter_context(tc.tile_pool(name="up", bufs=3))
        stripp = ctx.enter_context(tc.tile_pool(name="stripp", bufs=2))
        redp = ctx.enter_context(tc.tile_pool(name="redp", bufs=2))
        ptrp = ctx.enter_context(tc.tile_pool(name="ptr", bufs=2, space="PSUM"))
        pmmp = ctx.enter_context(tc.tile_pool(name="pmm", bufs=2, space="PSUM"))
        prop = ctx.enter_context(tc.tile_pool(name="pro", bufs=2, space="PSUM"))

        # ---- constants
        ident = cst.tile([P, P], F32)
        make_identity(nc, ident[:])
        gidx_sb = cst.tile([P, g.BT * 7], I32)
        nc.sync.dma_start(out=gidx_sb[:], in_=gidx[:, :])
        ridx_sb = cst.tile([P, g.AT * 6], I32)
        nc.sync.dma_start(out=ridx_sb[:], in_=ridx[:, :])
        bo_sb = cst.tile([P, len(g.KC)], F32)
        nc.sync.dma_start(out=bo_sb[:], in_=bo[:, :])

        def load_w(param, chunks, name):
            tiles = []
            for (o, k) in chunks:
                t = cst.tile([k, H], BF16, tag=f"{name}{o}")
                nc.sync.dma_start(out=t[:], in_=param[o:o + k, :])
                tiles.append(t)
            return tiles

        Wi_sb = load_w(Wi, g.FC, "wi")
        Wh_sb = load_w(Wh, g.KC, "wh")
        Wo1_sb = load_w(Wo1, g.AC, "wo1")
        Wo2_sb = load_w(Wo2, g.KC, "wo2")

        inp_res = inp_p.tile([P, g.BT, H], BF16)

        # ---- shared tail: msg (bf16 [P,H] row major) -> transpose -> @W -> u table
        def bond_tail(msg, W_sb, u_table, t):
            u_tab = u_table
            trow = t * P
            ptr = ptrp.tile([P, len(g.KC) * P], F32, tag="ptr")
            for ci, (o, k) in enumerate(g.KC):
                nc.tensor.transpose(out=ptr[0:k, ci * P:ci * P + P],
                                    in_=msg[:, o:o + k], identity=ident[:])
            lt = ltp.tile([P, len(g.KC), P], BF16, tag="lt")
            # copy PSUM->SBUF (only valid rows for the ragged last chunk)
            nfull = sum(1 for (_, k) in g.KC if k == P)
            if nfull:
                nc.scalar.copy(out=lt[:, 0:nfull, :], in_=ptr[:, 0:nfull * P])
            for ci, (o, k) in enumerate(g.KC):
                if k != P:
                    nc.scalar.copy(out=lt[0:k, ci, :], in_=ptr[0:k, ci * P:ci * P + P])
            pu = pmmp.tile([P, H], F32, tag="pu")
            for ci, (o, k) in enumerate(g.KC):
                nc.tensor.matmul(out=pu[:], lhsT=lt[0:k, ci, :], rhs=W_sb[ci][:],
                                 start=(ci == 0), stop=(ci == len(g.KC) - 1))
            u_sb = up.tile([P, H], BF16, tag="u")
            nc.scalar.copy(out=u_sb[:], in_=pu[:])
            nc.sync.dma_start(out=u_tab[trow:trow + P, :], in_=u_sb[:])

        # ---- pass 0: inp = f_bonds@Wi ; msg0 = relu(inp); u0
        FSTR = 8  # tiles per fbT strip load
        for t0 in range(0, g.BT, FSTR):
            nt = min(FSTR, g.BT - t0)
            fb_tiles = []
            for ci, (o, k) in enumerate(g.FC):
                ft = fbp.tile([k, FSTR * P], BF16, tag=f"fb{ci}")
                nc.sync.dma_start(out=ft[:, 0:nt * P], in_=fbT[o:o + k, t0 * P:(t0 + nt) * P])
                fb_tiles.append(ft)
            for ti in range(nt):
                t = t0 + ti
                pi = pmmp.tile([P, H], F32, tag="pi")
                for ci, (o, k) in enumerate(g.FC):
                    nc.tensor.matmul(out=pi[:], lhsT=fb_tiles[ci][:, ti * P:(ti + 1) * P],
                                     rhs=Wi_sb[ci][:],
                                     start=(ci == 0), stop=(ci == len(g.FC) - 1))
                nc.scalar.copy(out=inp_res[:, t, :], in_=pi[:])
                msg = msgp.tile([P, H], F32, tag="msg")
                nc.vector.tensor_scalar_max(out=msg[:], in0=pi[:], scalar1=0.0)
                bond_tail(msg, Wh_sb, Ul[0], t)

        ag_halves(0)

        # ---- passes 1..n_up: gather + update; last pass premultiplies W_o2
        for it in range(n_up):
            W_next = Wh_sb if it < n_up - 1 else Wo2_sb
            for t in range(g.BT):
                gt = gp.tile([P, 7, H], BF16, tag="gt")
                for kk in range(7):
                    nc.gpsimd.indirect_dma_start(
                        out=gt[:, kk, :], out_offset=None, in_=Uf[it][:, :],
                        in_offset=bass.IndirectOffsetOnAxis(
                            ap=gidx_sb[:, t * 7 + kk:t * 7 + kk + 1], axis=0))
                # tree sum into gt[:,0,:]: pairs (0,1)(2,3)(4,5) -> [0,1,2]
                nc.vector.tensor_tensor(out=gt[:, 0:3, :], in0=gt[:, 0:6:2, :],
                                        in1=gt[:, 1:7:2, :], op=AO.add)
                nc.vector.tensor_tensor(out=gt[:, 0, :], in0=gt[:, 0, :],
                                        in1=gt[:, 1, :], op=AO.add)
                nc.vector.tensor_tensor(out=gt[:, 0, :], in0=gt[:, 0, :],
                                        in1=gt[:, 2, :], op=AO.add)
                # minus reverse row: gt0 = (-1)*gt6 + gt0
                nc.vector.scalar_tensor_tensor(out=gt[:, 0, :], in0=gt[:, 6, :],
                                               scalar=-1.0, in1=gt[:, 0, :],
                                               op0=AO.mult, op1=AO.add)
                nc.vector.tensor_tensor(out=gt[:, 0, :], in0=gt[:, 0, :],
                                        in1=inp_res[:, t, :], op=AO.add)
                msg = msgp.tile([P, H], F32, tag="msg")
                nc.vector.tensor_scalar_max(out=msg[:], in0=gt[:, 0, :], scalar1=0.0)
                bond_tail(msg, W_next, Ul[it + 1], t)
            ag_halves(it + 1)

        # ---- readout pass over atoms
        Z = Uf[n_up]
        for grp in range(g.NG):
            strip = stripp.tile([P, len(g.KC), g.AG], F32, tag="strip")
            fa_tiles = []
            for ci, (o, k) in enumerate(g.AC):
                ft = fbp.tile([k, g.AG], BF16, tag=f"fa{ci}")
                nc.sync.dma_start(out=ft[:], in_=faT[o:o + k, grp * g.AG:(grp + 1) * g.AG])
                fa_tiles.append(ft)
            for ti in range(g.TPG):
                t = grp * g.TPG + ti
                rt = gp.tile([P, 6, H], BF16, tag="rt")
                for kk in range(6):
                    nc.gpsimd.indirect_dma_start(
                        out=rt[:, kk, :], out_offset=None, in_=Z[:, :],
                        in_offset=bass.IndirectOffsetOnAxis(
                            ap=ridx_sb[:, t * 6 + kk:t * 6 + kk + 1], axis=0))
                nc.vector.tensor_tensor(out=rt[:, 0:3, :], in0=rt[:, 0:5:2, :],
                                        in1=rt[:, 1:6:2, :], op=AO.add)
                nc.vector.tensor_tensor(out=rt[:, 0, :], in0=rt[:, 0, :],
                                        in1=rt[:, 1, :], op=AO.add)
                amsg = msgp.tile([P, H], F32, tag="amsg")
                nc.vector.tensor_tensor(out=amsg[:], in0=rt[:, 0, :],
                                        in1=rt[:, 2, :], op=AO.add)
                if debug:
                    nc.sync.dma_start(out=amsg_dbg[t * P:(t + 1) * P, :], in_=amsg[:])
                # transpose a_msg
                ptr = ptrp.tile([P, len(g.KC) * P], F32, tag="ptr")
                for ci, (o, k) in enumerate(g.KC):
                    nc.tensor.transpose(out=ptr[0:k, ci * P:ci * P + P],
                                        in_=amsg[:, o:o + k], identity=ident[:])
                lt = ltp.tile([P, len(g.KC), P], BF16, tag="lt")
                nfull = sum(1 for (_, k) in g.KC if k == P)
                if nfull:
                    nc.scalar.copy(out=lt[:, 0:nfull, :], in_=ptr[:, 0:nfull * P])
                for ci, (o, k) in enumerate(g.KC):
                    if k != P:
                        nc.scalar.copy(out=lt[0:k, ci, :], in_=ptr[0:k, ci * P:ci * P + P])
                # hT = relu(W_o1.T @ f_atomsT + amsgT + b_o)   (amsg already @W_o2)
                pro = prop.tile([P, len(g.KC), P], F32, tag="pro")
                hsum = up.tile([P, len(g.KC), P], F32, tag="hsum")
                for mi, (mo, mk) in enumerate(g.KC):
                    for ci, (o, k) in enumerate(g.AC):
                        nc.tensor.matmul(out=pro[0:mk, mi, :],
                                         lhsT=Wo1_sb[ci][:, mo:mo + mk],
                                         rhs=fa_tiles[ci][:, ti * P:(ti + 1) * P],
                                         start=(ci == 0), stop=(ci == len(g.AC) - 1))
                    nc.vector.tensor_tensor(out=hsum[0:mk, mi, :],
                                            in0=pro[0:mk, mi, :],
                                            in1=lt[0:mk, mi, :], op=AO.add)
                    nc.scalar.activation(out=strip[0:mk, mi, ti * P:(ti + 1) * P],
                                         in_=hsum[0:mk, mi, :],
                                         func=mybir.ActivationFunctionType.Relu,
                                         bias=bo_sb[0:mk, mi:mi + 1])
            if debug:
                for mi, (mo, mk) in enumerate(g.KC):
                    nc.sync.dma_start(out=ht_dbg[mi, 0:mk, grp * g.AG:(grp + 1) * g.AG],
                                      in_=strip[0:mk, mi, :])
            red = redp.tile([P, len(g.KC), g.MPG], F32, tag="red")
            for mi, (mo, mk) in enumerate(g.KC):
                nc.vector.reduce_sum(
                    out=red[0:mk, mi, :],
                    in_=strip[0:mk, mi, :].rearrange("p (m a) -> p m a", a=g.apm),
                    axis=mybir.AxisListType.X)
                nc.vector.tensor_scalar_mul(out=red[0:mk, mi, :], in0=red[0:mk, mi, :],
                                            scalar1=1.0 / g.apm)
                nc.sync.dma_start(out=outT[mi, 0:mk, grp * g.MPG:(grp + 1) * g.MPG],
                                  in_=red[0:mk, mi, :])
    return nc


# ---------------------------------------------------------------- host side
def _prep_core_inputs(g: Geo, c, f_atoms, f_bonds, a2b, b2a, b2revb, W_i, W_h, W_o, b_o):
    H = g.hidden
    bf = ml_dtypes.bfloat16
    b0, b1 = c * g.BS, (c + 1) * g.BS
    a0, a1 = c * g.AS, (c + 1) * g.AS

    fbT = np.zeros((g.bond_fd, g.BSP), dtype=bf)
    fbT[:, :g.BS] = f_bonds[b0:b1].T.astype(bf)
    faT = np.zeros((g.atom_fd, g.ASP), dtype=bf)
    faT[:, :g.AS] = f_atoms[a0:a1].T.astype(bf)

    # global row -> padded table row
    def tab(j):
        return (j // g.BS) * g.BSP + (j % g.BS)

    # bond gather indices: 6x a2b[b2a[j]] then b2revb[j]
    jj = np.arange(b0, b1)
    gi = np.empty((g.BSP, 7), dtype=np.int64)
    gi[:g.BS, 0:6] = a2b[b2a[jj]]
    gi[:g.BS, 6] = b2revb[jj]
    gi[g.BS:] = 0
    gi = tab(gi)
    # layout [P, BT*7]: col t*7+k, row p -> bond t*128+p
    gidx = gi.reshape(g.BT, P, 7).transpose(1, 0, 2).reshape(P, g.BT * 7).astype(np.int32)

    aa = np.arange(a0, a1)
    ri = np.zeros((g.ASP, 6), dtype=np.int64)
    ri[:g.AS] = a2b[aa]
    ri = tab(ri)
    ridx = ri.reshape(g.AT, P, 6).transpose(1, 0, 2).reshape(P, g.AT * 6).astype(np.int32)

    nk = len(g.KC)
    bo = np.zeros((P, nk), dtype=np.float32)
    for mi, (mo, mk) in enumerate(g.KC):
        bo[:mk, mi] = b_o[mo:mo + mk]

    return {
        "fbT": fbT, "faT": faT, "gidx": gidx, "ridx": ridx,
        "Wi": W_i.astype(bf), "Wh": W_h.astype(bf),
        "Wo1": W_o[:g.atom_fd].astype(bf), "Wo2": W_o[g.atom_fd:].astype(bf),
        "bo": bo,
    }


def _assemble_output(g: Geo, outs):
    H = g.hidden
    n_mols_core = g.mols_per_core
    res = np.empty((g.n_cores * n_mols_core, H), dtype=np.float32)
    for c, om in enumerate(outs):
        o = om["outT"]  # [nk, P, NG*MPG]
        flat = o.reshape(len(g.KC) * P, -1)
        rows = []
        for mi, (mo, mk) in enumerate(g.KC):
            rows.append(flat[mi * P:mi * P + mk])
        hT = np.concatenate(rows, axis=0)  # [H, NG*MPG]
        res[c * n_mols_core:(c + 1) * n_mols_core] = hT[:, :n_mols_core].T
    return res


def kernel(f_atoms, f_bonds, a2b, b2a, b2revb, mol_ids, W_i, W_h, W_o, b_o, _run=None):
    f_atoms = np.asarray(f_atoms, dtype=np.float32)
    f_bonds = np.asarray(f_bonds, dtype=np.float32)
    a2b = np.asarray(a2b, dtype=np.int64)
    b2a = np.asarray(b2a, dtype=np.int64)
    b2revb = np.asarray(b2revb, dtype=np.int64)
    W_i = np.asarray(W_i, dtype=np.float32)
    W_h = np.asarray(W_h, dtype=np.float32)
    W_o = np.asarray(W_o, dtype=np.float32)
    b_o = np.asarray(b_o, dtype=np.float32)

    g = Geo(n_atoms=f_atoms.shape[0], n_bonds=f_bonds.shape[0],
            hidden=W_h.shape[0], atom_fd=f_atoms.shape[1],
            bond_fd=f_bonds.shape[1], n_cores=8, apm=20)

    nc = build(g)
    in_maps = [_prep_core_inputs(g, c, f_atoms, f_bonds, a2b, b2a, b2revb,
                                 W_i, W_h, W_o, b_o) for c in range(g.n_cores)]
    if _run is None:
        split_waits(nc)
        res = run_bass_kernel_spmd(nc, in_maps, core_ids=list(range(g.n_cores)))
        outs = res.results
    else:
        outs = _run(nc, in_maps, g)
    return _assemble_output(g, outs)



# revision 11
# speedup vs baseline: 1.0693x; 1.0693x over previous
"""DMPNN encoder on 8 trn2 NeuronCores (Bass/Tile).

Formulation (exploits linearity of the neighbor-sum):
    inp   = f_bonds @ W_i                       (per-bond, row major)
    msg_0 = relu(inp)
    u_t   = msg_t @ W (W_h for t=0,1; W_o2 for t=2)   [per-bond shard, local]
    (allgather U_t across cores)
    msg_{t+1} = relu(inp + sum_k U_t[a2b[b2a[j],k]] - U_t[b2revb[j]])
    readout: a_msg = sum_k Z[a2b[a,k]];  hT = relu(W_o1.T@f_atomsT + W_o2.T@a_msgT + b_o)
    out = mean-pool hT over 20-atom molecules (transposed), host re-transposes.

Sharding: bonds and atoms split contiguously across 8 cores. All gathers are
indirect DMAs from allgathered (Shared DRAM) tables of premultiplied rows, so
the random a2b/b2a/b2revb indexing never crosses a core's local HBM.
"""
import sys
import types
import numpy as np
import ml_dtypes
from contextlib import ExitStack
from dataclasses import dataclass

import concourse.bass as bass
import concourse.mybir as mybir
import concourse.tile as tile
from concourse.masks import make_identity
from concourse.bass_utils import run_bass_kernel_spmd

P = 128
BF16 = mybir.dt.bfloat16
F32 = mybir.dt.float32
I32 = mybir.dt.int32
AO = mybir.AluOpType


# ---------------------------------------------------------------- wait split
_seq = [0]


def _mk_event(engine, waits, debug):
    _seq[0] += 1
    return mybir.InstEventSemaphore(
        name=f"WS-{_seq[0]}", engine=engine, ins=[], outs=[], debug=debug,
        sync_info=mybir.SyncInfo(on_wait=list(waits), on_update=[]),
    )


def split_waits(nc, cap_normal=1, cap_event=2):
    """This container's walrus caps sync waits at 1/inst (2 for
    EventSemaphore); Tile emits more. Move excess waits onto preceding
    EventSemaphore instructions on the same engine."""
    n_split = 0
    for fn in nc.m.functions:
        for blk in fn.blocks:
            insts = list(blk.instructions)
            out = []
            for inst in insts:
                si = inst.sync_info
                waits = list(si.on_wait) if (si is not None and si.on_wait) else []
                cap = cap_event if isinstance(inst, mybir.InstEventSemaphore) else cap_normal
                if len(waits) > cap:
                    extra, keep = waits[:-cap], waits[-cap:]
                    for i in range(0, len(extra), cap_event):
                        out.append(_mk_event(inst.engine, extra[i:i + cap_event], inst.debug))
                    inst.sync_info = mybir.SyncInfo(on_wait=keep, on_update=list(si.on_update or []))
                    n_split += 1
                out.append(inst)
            if len(out) != len(insts):
                blk.instructions = out
    return n_split


def install_ntff_shim():
    if "antenv.axon_hooks" in sys.modules:
        return
    mod = types.ModuleType("antenv.axon_hooks")
    state = {"hook": None}
    mod.set_axon_ntff_profile_hook = lambda h: state.__setitem__("hook", h)
    mod.get_axon_ntff_profile_hook = lambda: state["hook"]
    sys.modules["antenv.axon_hooks"] = mod
    import antenv
    antenv.axon_hooks = mod
    try:
        from trn_agent_boot.trn_boot import _ntff_profile_via_ctypes
        mod.set_axon_ntff_profile_hook(_ntff_profile_via_ctypes("/opt/axon/libaxon_pjrt.so"))
    except Exception:
        pass


# ---------------------------------------------------------------- geometry
@dataclass
class Geo:
    n_atoms: int
    n_bonds: int
    hidden: int
    atom_fd: int
    bond_fd: int
    n_cores: int
    apm: int  # atoms per mol

    def __post_init__(g):
        assert g.n_bonds % g.n_cores == 0 and g.n_atoms % g.n_cores == 0
        g.BS = g.n_bonds // g.n_cores
        g.AS = g.n_atoms // g.n_cores
        g.BT = -(-g.BS // P)            # bond tiles per core
        g.BSP = g.BT * P
        g.NCH = 4                       # AllGather chunks per pass
        assert g.BT % g.NCH == 0
        g.mols_per_core = g.AS // g.apm
        # atom group = lcm(P, apm) atoms so molecule boundaries align
        import math
        g.AG = math.lcm(P, g.apm)       # atoms per pool group
        g.TPG = g.AG // P               # tiles per group
        g.MPG = g.AG // g.apm           # mols per group
        g.NG = -(-g.AS // g.AG)         # groups per core
        g.AT = g.NG * g.TPG             # atom tiles per core (padded)
        g.ASP = g.AT * P
        # hidden chunks of <=128
        g.KC = []
        o = 0
        while o < g.hidden:
            g.KC.append((o, min(P, g.hidden - o)))
            o += P
        # bond_fd chunks
        g.FC = []
        o = 0
        while o < g.bond_fd:
            g.FC.append((o, min(P, g.bond_fd - o)))
            o += P
        # atom_fd chunks
        g.AC = []
        o = 0
        while o < g.atom_fd:
            g.AC.append((o, min(P, g.atom_fd - o)))
            o += P


DEFAULT_GEO = dict(n_atoms=100_000, n_bonds=200_000, hidden=300, atom_fd=133,
                   bond_fd=147, n_cores=8, apm=20)


# ---------------------------------------------------------------- device program
def build(g: Geo, depth: int = 3, debug: bool = False):
    H = g.hidden
    nc = bass.Bass(num_devices=g.n_cores)
    amsg_dbg = nc.dram_tensor("amsg_dbg", [g.ASP, H], BF16) if debug else None
    ht_dbg = nc.dram_tensor("ht_dbg", [len(g.KC), P, g.ASP], F32) if debug else None

    fbT = nc.declare_dram_parameter("fbT", [g.bond_fd, g.BSP], BF16, isOutput=False)
    faT = nc.declare_dram_parameter("faT", [g.atom_fd, g.ASP], BF16, isOutput=False)
    gidx = nc.declare_dram_parameter("gidx", [P, g.BT * 7], I32, isOutput=False)
    ridx = nc.declare_dram_parameter("ridx", [P, g.AT * 6], I32, isOutput=False)
    Wi = nc.declare_dram_parameter("Wi", [g.bond_fd, H], BF16, isOutput=False)
    Wh = nc.declare_dram_parameter("Wh", [H, H], BF16, isOutput=False)
    Wo1 = nc.declare_dram_parameter("Wo1", [g.atom_fd, H], BF16, isOutput=False)
    Wo2 = nc.declare_dram_parameter("Wo2", [H, H], BF16, isOutput=False)
    bo = nc.declare_dram_parameter("bo", [P, len(g.KC)], F32, isOutput=False)
    outT = nc.declare_dram_parameter("outT", [len(g.KC), P, g.NG * g.MPG], F32, isOutput=True)

    n_up = depth - 1  # message update rounds (gather passes)
    # local premultiplied tables (per AG chunk) + allgathered Shared versions.
    # Uf row layout: [NCH, n_cores, CS] chunk-major so each chunk's AllGather
    # writes one contiguous range; host index prep matches.
    NCH = g.NCH
    CS = g.BSP // NCH
    TC = g.BT // NCH  # tiles per chunk
    Ul = [[nc.dram_tensor(f"U{t}l{ch}", [CS, H], BF16) for ch in range(NCH)]
          for t in range(n_up + 1)]
    Uf = [nc.dram_tensor(f"U{t}f", [g.n_cores * g.BSP, H], BF16, addr_space="Shared")
          for t in range(n_up + 1)]

    def ag_chunk(t, ch):
        nc.gpsimd.collective_compute(
            "AllGather", AO.bypass, replica_groups=rg,
            ins=[Ul[t][ch][:, :].opt()],
            outs=[Uf[t][ch * g.n_cores * CS:(ch + 1) * g.n_cores * CS, :].opt()])

    rg = [list(range(g.n_cores))]

    with tile.TileContext(nc) as tc, ExitStack() as ctx:
        cst = ctx.enter_context(tc.tile_pool(name="cst", bufs=1))
        inp_p = ctx.enter_context(tc.tile_pool(name="inp", bufs=1))
        fbp = ctx.enter_context(tc.tile_pool(name="fbp", bufs=2))
        gp = ctx.enter_context(tc.tile_pool(name="gp", bufs=4))
        msgp = ctx.enter_context(tc.tile_pool(name="msgp", bufs=3))
        ltp = ctx.enter_context(tc.tile_pool(name="ltp", bufs=3))
        up = ctx.enter_context(tc.tile_pool(name="up", bufs=3))
        stripp = ctx.enter_context(tc.tile_pool(name="stripp", bufs=2))
        redp = ctx.enter_context(tc.tile_pool(name="redp", bufs=2))
        ptrp = ctx.enter_context(tc.tile_pool(name="ptr", bufs=2, space="PSUM"))
        pmmp = ctx.enter_context(tc.tile_pool(name="pmm", bufs=2, space="PSUM"))
        prop = ctx.enter_context(tc.tile_pool(name="pro", bufs=2, space="PSUM"))

        # ---- constants
        ident = cst.tile([P, P], F32)
        make_identity(nc, ident[:])
        gidx_sb = cst.tile([P, g.BT * 7], I32)
        nc.sync.dma_start(out=gidx_sb[:], in_=gidx[:, :])
        ridx_sb = cst.tile([P, g.AT * 6], I32)
        nc.sync.dma_start(out=ridx_sb[:], in_=ridx[:, :])
        bo_sb = cst.tile([P, len(g.KC)], F32)
        nc.sync.dma_start(out=bo_sb[:], in_=bo[:, :])

        def load_w(param, chunks, name):
            tiles = []
            for (o, k) in chunks:
                t = cst.tile([k, H], BF16, tag=f"{name}{o}")
                nc.sync.dma_start(out=t[:], in_=param[o:o + k, :])
                tiles.append(t)
            return tiles

        Wi_sb = load_w(Wi, g.FC, "wi")
        Wh_sb = load_w(Wh, g.KC, "wh")
        Wo1_sb = load_w(Wo1, g.AC, "wo1")
        Wo2_sb = load_w(Wo2, g.KC, "wo2")

        inp_res = inp_p.tile([P, g.BT, H], BF16)

        # ---- shared tail: msg (bf16 [P,H] row major) -> transpose -> @W -> u table
        def bond_tail(msg, u_tables, W_sb, t):
            u_tab = u_tables[t // TC]
            trow = (t % TC) * P
            ptr = ptrp.tile([P, len(g.KC) * P], F32, tag="ptr")
            for ci, (o, k) in enumerate(g.KC):
                nc.tensor.transpose(out=ptr[0:k, ci * P:ci * P + P],
                                    in_=msg[:, o:o + k], identity=ident[:])
            lt = ltp.tile([P, len(g.KC), P], BF16, tag="lt")
            # copy PSUM->SBUF (only valid rows for the ragged last chunk)
            nfull = sum(1 for (_, k) in g.KC if k == P)
            if nfull:
                nc.scalar.copy(out=lt[:, 0:nfull, :], in_=ptr[:, 0:nfull * P])
            for ci, (o, k) in enumerate(g.KC):
                if k != P:
                    nc.scalar.copy(out=lt[0:k, ci, :], in_=ptr[0:k, ci * P:ci * P + P])
            pu = pmmp.tile([P, H], F32, tag="pu")
            for ci, (o, k) in enumerate(g.KC):
                nc.tensor.matmul(out=pu[:], lhsT=lt[0:k, ci, :], rhs=W_sb[ci][:],
                                 start=(ci == 0), stop=(ci == len(g.KC) - 1))
            u_sb = up.tile([P, H], BF16, tag="u")
            nc.scalar.copy(out=u_sb[:], in_=pu[:])
            nc.sync.dma_start(out=u_tab[trow:trow + P, :], in_=u_sb[:])

        # ---- pass 0: inp = f_bonds@Wi ; msg0 = relu(inp); u0
        FSTR = 8  # tiles per fbT strip load
        for t0 in range(0, g.BT, FSTR):
            nt = min(FSTR, g.BT - t0)
            fb_tiles = []
            for ci, (o, k) in enumerate(g.FC):
                ft = fbp.tile([k, FSTR * P], BF16, tag=f"fb{ci}")
                nc.sync.dma_start(out=ft[:, 0:nt * P], in_=fbT[o:o + k, t0 * P:(t0 + nt) * P])
                fb_tiles.append(ft)
            for ti in range(nt):
                t = t0 + ti
                pi = pmmp.tile([P, H], F32, tag="pi")
                for ci, (o, k) in enumerate(g.FC):
                    nc.tensor.matmul(out=pi[:], lhsT=fb_tiles[ci][:, ti * P:(ti + 1) * P],
                                     rhs=Wi_sb[ci][:],
                                     start=(ci == 0), stop=(ci == len(g.FC) - 1))
                nc.scalar.copy(out=inp_res[:, t, :], in_=pi[:])
                msg = msgp.tile([P, H], F32, tag="msg")
                nc.vector.tensor_scalar_max(out=msg[:], in0=pi[:], scalar1=0.0)
                bond_tail(msg, Ul[0], Wh_sb, t)
                if (t + 1) % TC == 0:
                    ag_chunk(0, t // TC)

        # ---- passes 1..n_up: gather + update; last pass premultiplies W_o2
        for it in range(n_up):
            W_next = Wh_sb if it < n_up - 1 else Wo2_sb
            for t in range(g.BT):
                gt = gp.tile([P, 7, H], BF16, tag="gt")
                for kk in range(7):
                    nc.gpsimd.indirect_dma_start(
                        out=gt[:, kk, :], out_offset=None, in_=Uf[it][:, :],
                        in_offset=bass.IndirectOffsetOnAxis(
                            ap=gidx_sb[:, t * 7 + kk:t * 7 + kk + 1], axis=0))
                # tree sum into gt[:,0,:]: pairs (0,1)(2,3)(4,5) -> [0,1,2]
                nc.vector.tensor_tensor(out=gt[:, 0:3, :], in0=gt[:, 0:6:2, :],
                                        in1=gt[:, 1:7:2, :], op=AO.add)
                nc.vector.tensor_tensor(out=gt[:, 0, :], in0=gt[:, 0, :],
                                        in1=gt[:, 1, :], op=AO.add)
                nc.vector.tensor_tensor(out=gt[:, 0, :], in0=gt[:, 0, :],
                                        in1=gt[:, 2, :], op=AO.add)
                # minus reverse row: gt0 = (-1)*gt6 + gt0
                nc.vector.scalar_tensor_tensor(out=gt[:, 0, :], in0=gt[:, 6, :],
                                               scalar=-1.0, in1=gt[:, 0, :],
                                               op0=AO.mult, op1=AO.add)
                nc.vector.tensor_tensor(out=gt[:, 0, :], in0=gt[:, 0, :],
                                        in1=inp_res[:, t, :], op=AO.add)
                msg = msgp.tile([P, H], F32, tag="msg")
                nc.vector.tensor_scalar_max(out=msg[:], in0=gt[:, 0, :], scalar1=0.0)
                bond_tail(msg, Ul[it + 1], W_next, t)
                if (t + 1) % TC == 0:
                    ag_chunk(it + 1, t // TC)

        # ---- readout pass over atoms
        Z = Uf[n_up]
        for grp in range(g.NG):
            strip = stripp.tile([P, len(g.KC), g.AG], F32, tag="strip")
            fa_tiles = []
            for ci, (o, k) in enumerate(g.AC):
                ft = fbp.tile([k, g.AG], BF16, tag=f"fa{ci}")
                nc.sync.dma_start(out=ft[:], in_=faT[o:o + k, grp * g.AG:(grp + 1) * g.AG])
                fa_tiles.append(ft)
            for ti in range(g.TPG):
                t = grp * g.TPG + ti
                rt = gp.tile([P, 6, H], BF16, tag="rt")
                for kk in range(6):
                    nc.gpsimd.indirect_dma_start(
                        out=rt[:, kk, :], out_offset=None, in_=Z[:, :],
                        in_offset=bass.IndirectOffsetOnAxis(
                            ap=ridx_sb[:, t * 6 + kk:t * 6 + kk + 1], axis=0))
                nc.vector.tensor_tensor(out=rt[:, 0:3, :], in0=rt[:, 0:5:2, :],
                                        in1=rt[:, 1:6:2, :], op=AO.add)
                nc.vector.tensor_tensor(out=rt[:, 0, :], in0=rt[:, 0, :],
                                        in1=rt[:, 1, :], op=AO.add)
                amsg = msgp.tile([P, H], F32, tag="amsg")
                nc.vector.tensor_tensor(out=amsg[:], in0=rt[:, 0, :],
                                        in1=rt[:, 2, :], op=AO.add)
                if debug:
                    nc.sync.dma_start(out=amsg_dbg[t * P:(t + 1) * P, :], in_=amsg[:])
                # transpose a_msg
                ptr = ptrp.tile([P, len(g.KC) * P], F32, tag="ptr")
                for ci, (o, k) in enumerate(g.KC):
                    nc.tensor.transpose(out=ptr[0:k, ci * P:ci * P + P],
                                        in_=amsg[:, o:o + k], identity=ident[:])
                lt = ltp.tile([P, len(g.KC), P], BF16, tag="lt")
                nfull = sum(1 for (_, k) in g.KC if k == P)
                if nfull:
                    nc.scalar.copy(out=lt[:, 0:nfull, :], in_=ptr[:, 0:nfull * P])
                for ci, (o, k) in enumerate(g.KC):
                    if k != P:
                        nc.scalar.copy(out=lt[0:k, ci, :], in_=ptr[0:k, ci * P:ci * P + P])
                # hT = relu(W_o1.T @ f_atomsT + amsgT + b_o)   (amsg already @W_o2)
                pro = prop.tile([P, len(g.KC), P], F32, tag="pro")
                hsum = up.tile([P, len(g.KC), P], F32, tag="hsum")
                for mi, (mo, mk) in enumerate(g.KC):
                    for ci, (o, k) in enumerate(g.AC):
                        nc.tensor.matmul(out=pro[0:mk, mi, :],
                                         lhsT=Wo1_sb[ci][:, mo:mo + mk],
                                         rhs=fa_tiles[ci][:, ti * P:(ti + 1) * P],
                                         start=(ci == 0), stop=(ci == len(g.AC) - 1))
                    nc.vector.tensor_tensor(out=hsum[0:mk, mi, :],
                                            in0=pro[0:mk, mi, :],
                                            in1=lt[0:mk, mi, :], op=AO.add)
                    nc.scalar.activation(out=strip[0:mk, mi, ti * P:(ti + 1) * P],
                                         in_=hsum[0:mk, mi, :],
                                         func=mybir.ActivationFunctionType.Relu,
                                         bias=bo_sb[0:mk, mi:mi + 1])
            if debug:
                for mi, (mo, mk) in enumerate(g.KC):
                    nc.sync.dma_start(out=ht_dbg[mi, 0:mk, grp * g.AG:(grp + 1) * g.AG],
                                      in_=strip[0:mk, mi, :])
            red = redp.tile([P, len(g.KC), g.MPG], F32, tag="red")
            for mi, (mo, mk) in enumerate(g.KC):
                nc.vector.reduce_sum(
                    out=red[0:mk, mi, :],
                    in_=strip[0:mk, mi, :].rearrange("p (m a) -> p m a", a=g.apm),
                    axis=mybir.AxisListType.X)
                nc.vector.tensor_scalar_mul(out=red[0:mk, mi, :], in0=red[0:mk, mi, :],
                                            scalar1=1.0 / g.apm)
                nc.sync.dma_start(out=outT[mi, 0:mk, grp * g.MPG:(grp + 1) * g.MPG],
                                  in_=red[0:mk, mi, :])
    return nc


# ---------------------------------------------------------------- host side
def _prep_core_inputs(g: Geo, c, f_atoms, f_bonds, a2b, b2a, b2revb, W_i, W_h, W_o, b_o):
    H = g.hidden
    bf = ml_dtypes.bfloat16
    b0, b1 = c * g.BS, (c + 1) * g.BS
    a0, a1 = c * g.AS, (c + 1) * g.AS

    fbT = np.zeros((g.bond_fd, g.BSP), dtype=bf)
    fbT[:, :g.BS] = f_bonds[b0:b1].T.astype(bf)
    faT = np.zeros((g.atom_fd, g.ASP), dtype=bf)
    faT[:, :g.AS] = f_atoms[a0:a1].T.astype(bf)

    # global row -> padded table row (chunk-major AllGather layout:
    # [NCH, n_cores, CS] with CS = BSP//NCH rows per core per chunk)
    CS = g.BSP // g.NCH

    def tab(j):
        c, r = j // g.BS, j % g.BS
        return (r // CS) * (g.n_cores * CS) + c * CS + (r % CS)

    # bond gather indices: 6x a2b[b2a[j]] then b2revb[j]
    jj = np.arange(b0, b1)
    gi = np.empty((g.BSP, 7), dtype=np.int64)
    gi[:g.BS, 0:6] = a2b[b2a[jj]]
    gi[:g.BS, 6] = b2revb[jj]
    gi[g.BS:] = 0
    gi = tab(gi)
    # layout [P, BT*7]: col t*7+k, row p -> bond t*128+p
    gidx = gi.reshape(g.BT, P, 7).transpose(1, 0, 2).reshape(P, g.BT * 7).astype(np.int32)

    aa = np.arange(a0, a1)
    ri = np.zeros((g.ASP, 6), dtype=np.int64)
    ri[:g.AS] = a2b[aa]
    ri = tab(ri)
    ridx = ri.reshape(g.AT, P, 6).transpose(1, 0, 2).reshape(P, g.AT * 6).astype(np.int32)

    nk = len(g.KC)
    bo = np.zeros((P, nk), dtype=np.float32)
    for mi, (mo, mk) in enumerate(g.KC):
        bo[:mk, mi] = b_o[mo:mo + mk]

    return {
        "fbT": fbT, "faT": faT, "gidx": gidx, "ridx": ridx,
        "Wi": W_i.astype(bf), "Wh": W_h.astype(bf),
        "Wo1": W_o[:g.atom_fd].astype(bf), "Wo2": W_o[g.atom_fd:].astype(bf),
        "bo": bo,
    }


def _assemble_output(g: Geo, outs):
    H = g.hidden
    n_mols_core = g.mols_per_core
    res = np.empty((g.n_cores * n_mols_core, H), dtype=np.float32)
    for c, om in enumerate(outs):
        o = om["outT"]  # [nk, P, NG*MPG]
        flat = o.reshape(len(g.KC) * P, -1)
        rows = []
        for mi, (mo, mk) in enumerate(g.KC):
            rows.append(flat[mi * P:mi * P + mk])
        hT = np.concatenate(rows, axis=0)  # [H, NG*MPG]
        res[c * n_mols_core:(c + 1) * n_mols_core] = hT[:, :n_mols_core].T
    return res


def kernel(f_atoms, f_bonds, a2b, b2a, b2revb, mol_ids, W_i, W_h, W_o, b_o, _run=None):
    f_atoms = np.asarray(f_atoms, dtype=np.float32)
    f_bonds = np.asarray(f_bonds, dtype=np.float32)
    a2b = np.asarray(a2b, dtype=np.int64)
    b2a = np.asarray(b2a, dtype=np.int64)
    b2revb = np.asarray(b2revb, dtype=np.int64)
    W_i = np.asarray(W_i, dtype=np.float32)
    W_h = np.asarray(W_h, dtype=np.float32)
    W_o = np.asarray(W_o, dtype=np.float32)
    b_o = np.asarray(b_o, dtype=np.float32)

    g = Geo(n_atoms=f_atoms.shape[0], n_bonds=f_bonds.shape[0],
            hidden=W_h.shape[0], atom_fd=f_atoms.shape[1],
            bond_fd=f_bonds.shape[1], n_cores=8, apm=20)

    nc = build(g)
    in_maps = [_prep_core_inputs(g, c, f_atoms, f_bonds, a2b, b2a, b2revb,
                                 W_i, W_h, W_o, b_o) for c in range(g.n_cores)]
    if _run is None:
        split_waits(nc)
        res = run_bass_kernel_spmd(nc, in_maps, core_ids=list(range(g.n_cores)))
        outs = res.results
    else:
        outs = _run(nc, in_maps, g)
    return _assemble_output(g, outs)



# revision 23
# speedup vs baseline: 1.2223x; 1.1431x over previous
"""DMPNN encoder on 8 trn2 NeuronCores (Bass/Tile).

Per-atom formulation (dedups neighbor-sum across bonds sharing a source atom):
    msg_0 = relu(inp),  inp = f_bonds @ W_i            [bond rows, raw bf16]
    (allgather msg_t chunks across cores -> Mf)
    A_t[a]  = sum_k Mf_t[a2b[a,k]]                     [atom rows; 6 gathers/tile]
    (allgather A_t chunks -> Af;  t=0,1 only)
    m_j     = Af_t[b2a[j]] - Mf_t[b2revb[j]]           [2 gathers/bond tile]
    msg_{t+1} = relu(inp + m @ W_h)                    [transpose + matmul on-tile]
    readout: amsg = A_2 (local, no AG); hT = relu(W_o1.T@f_atomsT + (amsg@W_o2)T + b_o)
    out = mean-pool hT over 20-atom molecules (transposed), host re-transposes.

Tables are raw bf16 messages; premultiply by W_h happens after the gather-diff,
which is what lets one A row serve every bond with the same source atom.
AllGathers are split into NCH chunks emitted as soon as their tiles finish, so
only the last chunk sits on the critical path.
"""
import sys
import types
import numpy as np
import ml_dtypes
from contextlib import ExitStack
from dataclasses import dataclass

import concourse.bass as bass
import concourse.mybir as mybir
import concourse.tile as tile
from concourse.masks import make_identity
from concourse.bass_utils import run_bass_kernel_spmd

P = 128
BF16 = mybir.dt.bfloat16
F32 = mybir.dt.float32
I32 = mybir.dt.int32
AO = mybir.AluOpType


# ---------------------------------------------------------------- wait split
_seq = [0]


def _mk_event(engine, waits, debug):
    _seq[0] += 1
    return mybir.InstEventSemaphore(
        name=f"WS-{_seq[0]}", engine=engine, ins=[], outs=[], debug=debug,
        sync_info=mybir.SyncInfo(on_wait=list(waits), on_update=[]),
    )


def split_waits(nc, cap_normal=1, cap_event=2):
    """This container's walrus caps sync waits at 1/inst (2 for
    EventSemaphore); Tile emits more. Move excess waits onto preceding
    EventSemaphore instructions on the same engine."""
    n_split = 0
    for fn in nc.m.functions:
        for blk in fn.blocks:
            insts = list(blk.instructions)
            out = []
            for inst in insts:
                si = inst.sync_info
                waits = list(si.on_wait) if (si is not None and si.on_wait) else []
                cap = cap_event if isinstance(inst, mybir.InstEventSemaphore) else cap_normal
                if len(waits) > cap:
                    extra, keep = waits[:-cap], waits[-cap:]
                    for i in range(0, len(extra), cap_event):
                        out.append(_mk_event(inst.engine, extra[i:i + cap_event], inst.debug))
                    inst.sync_info = mybir.SyncInfo(on_wait=keep, on_update=list(si.on_update or []))
                    n_split += 1
                out.append(inst)
            if len(out) != len(insts):
                blk.instructions = out
    return n_split


def install_ntff_shim():
    if "antenv.axon_hooks" in sys.modules:
        return
    mod = types.ModuleType("antenv.axon_hooks")
    state = {"hook": None}
    mod.set_axon_ntff_profile_hook = lambda h: state.__setitem__("hook", h)
    mod.get_axon_ntff_profile_hook = lambda: state["hook"]
    sys.modules["antenv.axon_hooks"] = mod
    import antenv
    antenv.axon_hooks = mod
    try:
        from trn_agent_boot.trn_boot import _ntff_profile_via_ctypes
        mod.set_axon_ntff_profile_hook(_ntff_profile_via_ctypes("/opt/axon/libaxon_pjrt.so"))
    except Exception:
        pass


# ---------------------------------------------------------------- geometry
@dataclass
class Geo:
    n_atoms: int
    n_bonds: int
    hidden: int
    atom_fd: int
    bond_fd: int
    n_cores: int
    apm: int  # atoms per mol

    def __post_init__(g):
        assert g.n_bonds % g.n_cores == 0 and g.n_atoms % g.n_cores == 0
        g.BS = g.n_bonds // g.n_cores
        g.AS = g.n_atoms // g.n_cores
        g.BT = -(-g.BS // P)            # bond tiles per core
        g.BSP = g.BT * P
        g.NCHB = 7                      # AllGather chunks (bond tables)
        g.NCHA = 5                      # AllGather chunks (atom tables)
        assert g.BT % g.NCHB == 0
        g.mols_per_core = g.AS // g.apm
        # atom group = lcm(P, apm) atoms so molecule boundaries align
        import math
        g.AG = math.lcm(P, g.apm)       # atoms per pool group
        g.TPG = g.AG // P               # tiles per group
        g.MPG = g.AG // g.apm           # mols per group
        g.NG = -(-g.AS // g.AG)         # groups per core
        g.AT = g.NG * g.TPG             # atom tiles per core (padded)
        g.ASP = g.AT * P
        assert g.AT % g.NCHA == 0
        # hidden chunks of <=128
        g.KC = []
        o = 0
        while o < g.hidden:
            g.KC.append((o, min(P, g.hidden - o)))
            o += P
        # bond_fd chunks
        g.FC = []
        o = 0
        while o < g.bond_fd:
            g.FC.append((o, min(P, g.bond_fd - o)))
            o += P
        # atom_fd chunks
        g.AC = []
        o = 0
        while o < g.atom_fd:
            g.AC.append((o, min(P, g.atom_fd - o)))
            o += P


DEFAULT_GEO = dict(n_atoms=100_000, n_bonds=200_000, hidden=300, atom_fd=133,
                   bond_fd=147, n_cores=8, apm=20)


# ---------------------------------------------------------------- device program
def build(g: Geo, depth: int = 3):
    H = g.hidden
    nc = bass.Bass(num_devices=g.n_cores)

    fbT = nc.declare_dram_parameter("fbT", [g.bond_fd, g.BSP], BF16, isOutput=False)
    faT = nc.declare_dram_parameter("faT", [g.atom_fd, g.ASP], BF16, isOutput=False)
    aidx = nc.declare_dram_parameter("aidx", [P, g.AT * 6], I32, isOutput=False)
    bidx = nc.declare_dram_parameter("bidx", [P, g.BT * 2], I32, isOutput=False)
    Wi = nc.declare_dram_parameter("Wi", [g.bond_fd, H], BF16, isOutput=False)
    Wh = nc.declare_dram_parameter("Wh", [H, H], BF16, isOutput=False)
    Wo1 = nc.declare_dram_parameter("Wo1", [g.atom_fd, H], BF16, isOutput=False)
    Wo2 = nc.declare_dram_parameter("Wo2", [H, H], BF16, isOutput=False)
    bo = nc.declare_dram_parameter("bo", [P, len(g.KC)], F32, isOutput=False)
    outT = nc.declare_dram_parameter("outT", [len(g.KC), P, g.NG * g.MPG], F32, isOutput=True)

    n_up = depth - 1  # message update rounds
    CSB = g.BSP // g.NCHB   # bond rows per AG chunk
    TCB = g.BT // g.NCHB    # bond tiles per chunk
    CSA = g.ASP // g.NCHA   # atom rows per AG chunk
    TCA = g.AT // g.NCHA    # atom tiles per chunk
    # raw message tables: local chunks + allgathered Shared
    Ml = [[nc.dram_tensor(f"M{t}l{c}", [CSB, H], BF16) for c in range(g.NCHB)]
          for t in range(n_up + 1)]
    Mf = [nc.dram_tensor(f"M{t}f", [g.n_cores * g.BSP, H], BF16, addr_space="Shared")
          for t in range(n_up + 1)]
    # per-atom sum tables (t=0..n_up-1 need AG; last is consumed locally)
    Al = [[nc.dram_tensor(f"A{t}l{c}", [CSA, H], BF16) for c in range(g.NCHA)]
          for t in range(n_up)]
    Af = [nc.dram_tensor(f"A{t}f", [g.n_cores * g.ASP, H], BF16, addr_space="Shared")
          for t in range(n_up)]

    rg = [list(range(g.n_cores))]

    def ag_m(t, ch):
        nc.gpsimd.collective_compute(
            "AllGather", AO.bypass, replica_groups=rg,
            ins=[Ml[t][ch][:, :].opt()],
            outs=[Mf[t][ch * g.n_cores * CSB:(ch + 1) * g.n_cores * CSB, :].opt()])

    def ag_a(t, ch):
        nc.gpsimd.collective_compute(
            "AllGather", AO.bypass, replica_groups=rg,
            ins=[Al[t][ch][:, :].opt()],
            outs=[Af[t][ch * g.n_cores * CSA:(ch + 1) * g.n_cores * CSA, :].opt()])

    with tile.TileContext(nc) as tc, ExitStack() as ctx:
        cst = ctx.enter_context(tc.tile_pool(name="cst", bufs=1))
        inp_p = ctx.enter_context(tc.tile_pool(name="inp", bufs=1))
        fbp = ctx.enter_context(tc.tile_pool(name="fbp", bufs=2))
        gp = ctx.enter_context(tc.tile_pool(name="gp", bufs=4))
        msgp = ctx.enter_context(tc.tile_pool(name="msgp", bufs=3))
        ltp = ctx.enter_context(tc.tile_pool(name="ltp", bufs=3))
        up = ctx.enter_context(tc.tile_pool(name="up", bufs=3))
        stripp = ctx.enter_context(tc.tile_pool(name="stripp", bufs=2))
        redp = ctx.enter_context(tc.tile_pool(name="redp", bufs=2))
        ptrp = ctx.enter_context(tc.tile_pool(name="ptr", bufs=2, space="PSUM"))
        pmmp = ctx.enter_context(tc.tile_pool(name="pmm", bufs=2, space="PSUM"))
        prop = ctx.enter_context(tc.tile_pool(name="pro", bufs=2, space="PSUM"))

        # ---- constants
        ident = cst.tile([P, P], F32)
        make_identity(nc, ident[:])
        identb = cst.tile([P, P], BF16)
        nc.scalar.copy(out=identb[:], in_=ident[:])
        aidx_sb = cst.tile([P, g.AT * 6], I32)
        nc.sync.dma_start(out=aidx_sb[:], in_=aidx[:, :])
        bidx_sb = cst.tile([P, g.BT * 2], I32)
        nc.sync.dma_start(out=bidx_sb[:], in_=bidx[:, :])
        bo_sb = cst.tile([P, len(g.KC)], F32)
        nc.sync.dma_start(out=bo_sb[:], in_=bo[:, :])

        def load_w(param, chunks, name):
            tiles = []
            for (o, k) in chunks:
                t = cst.tile([k, H], BF16, tag=f"{name}{o}")
                nc.sync.dma_start(out=t[:], in_=param[o:o + k, :])
                tiles.append(t)
            return tiles

        Wi_sb = load_w(Wi, g.FC, "wi")
        Wh_sb = load_w(Wh, g.KC, "wh")
        Wo1_sb = load_w(Wo1, g.AC, "wo1")
        Wo2_sb = load_w(Wo2, g.KC, "wo2")

        inp_res = inp_p.tile([P, g.BT, H], BF16)

        def store_msg(t, ti, msg_sb):
            u_tab = Ml[t][ti // TCB]
            nc.sync.dma_start(out=u_tab[(ti % TCB) * P:(ti % TCB) * P + P, :],
                              in_=msg_sb[:])

        # transpose [P, H] -> lt [k-chunks, P] (bf16, via PE + PSUM)
        def transpose_to_lt(src):
            idn = ident if src.dtype == F32 else identb
            ptr = ptrp.tile([P, len(g.KC) * P], src.dtype, tag=f"ptr_{src.dtype}")
            for ci, (o, k) in enumerate(g.KC):
                nc.tensor.transpose(out=ptr[0:k, ci * P:ci * P + P],
                                    in_=src[:, o:o + k], identity=idn[:])
            lt = ltp.tile([P, len(g.KC), P], BF16, tag="lt")
            nfull = sum(1 for (_, k) in g.KC if k == P)
            if nfull:
                nc.scalar.copy(out=lt[:, 0:nfull, :], in_=ptr[:, 0:nfull * P])
            for ci, (o, k) in enumerate(g.KC):
                if k != P:
                    nc.scalar.copy(out=lt[0:k, ci, :], in_=ptr[0:k, ci * P:ci * P + P])
            return lt

        # ---- pass 0: inp = f_bonds@Wi ; msg0 = relu(inp)
        FSTR = 8  # tiles per fbT strip load
        for t0 in range(0, g.BT, FSTR):
            nt = min(FSTR, g.BT - t0)
            fb_tiles = []
            for ci, (o, k) in enumerate(g.FC):
                ft = fbp.tile([k, FSTR * P], BF16, tag=f"fb{ci}")
                nc.sync.dma_start(out=ft[:, 0:nt * P], in_=fbT[o:o + k, t0 * P:(t0 + nt) * P])
                fb_tiles.append(ft)
            for ti in range(nt):
                t = t0 + ti
                pi = pmmp.tile([P, H], F32, tag="pm")
                for ci, (o, k) in enumerate(g.FC):
                    nc.tensor.matmul(out=pi[:], lhsT=fb_tiles[ci][:, ti * P:(ti + 1) * P],
                                     rhs=Wi_sb[ci][:],
                                     start=(ci == 0), stop=(ci == len(g.FC) - 1))
                nc.scalar.copy(out=inp_res[:, t, :], in_=pi[:])
                msg = msgp.tile([P, H], BF16, tag="msg")
                nc.vector.tensor_scalar_max(out=msg[:], in0=pi[:], scalar1=0.0)
                store_msg(0, t, msg)
                if (t + 1) % TCB == 0:
                    ag_m(0, t // TCB)

        # ---- rounds t = 0..n_up-1:
        #   A_t = atom sums of Mf[t]; (AG A_t); msg_{t+1} = relu(inp + (A-rev)@Wh)
        for it in range(n_up):
            # A-build over this core's atoms
            for t in range(g.AT):
                gt = gp.tile([P, 6, H], BF16, tag="gt")
                for kk in range(6):
                    nc.gpsimd.indirect_dma_start(
                        out=gt[:, kk, :], out_offset=None, in_=Mf[it][:, :],
                        in_offset=bass.IndirectOffsetOnAxis(
                            ap=aidx_sb[:, t * 6 + kk:t * 6 + kk + 1], axis=0))
                nc.vector.tensor_tensor(out=gt[:, 0:3, :], in0=gt[:, 0:5:2, :],
                                        in1=gt[:, 1:6:2, :], op=AO.add)
                nc.vector.tensor_tensor(out=gt[:, 0, :], in0=gt[:, 0, :],
                                        in1=gt[:, 1, :], op=AO.add)
                asum = msgp.tile([P, H], BF16, tag="asum")
                nc.vector.tensor_tensor(out=asum[:], in0=gt[:, 0, :],
                                        in1=gt[:, 2, :], op=AO.add)
                a_tab = Al[it][t // TCA]
                nc.sync.dma_start(out=a_tab[(t % TCA) * P:(t % TCA) * P + P, :],
                                  in_=asum[:])
                if (t + 1) % TCA == 0:
                    ag_a(it, t // TCA)

            # bond update
            for t in range(g.BT):
                gb = gp.tile([P, 2, H], BF16, tag="gb")
                nc.gpsimd.indirect_dma_start(
                    out=gb[:, 0, :], out_offset=None, in_=Af[it][:, :],
                    in_offset=bass.IndirectOffsetOnAxis(
                        ap=bidx_sb[:, t * 2:t * 2 + 1], axis=0))
                nc.gpsimd.indirect_dma_start(
                    out=gb[:, 1, :], out_offset=None, in_=Mf[it][:, :],
                    in_offset=bass.IndirectOffsetOnAxis(
                        ap=bidx_sb[:, t * 2 + 1:t * 2 + 2], axis=0))
                diff = msgp.tile([P, H], BF16, tag="diff")
                nc.vector.scalar_tensor_tensor(out=diff[:], in0=gb[:, 1, :],
                                               scalar=-1.0, in1=gb[:, 0, :],
                                               op0=AO.mult, op1=AO.add)
                lt = transpose_to_lt(diff)
                pm = pmmp.tile([P, H], F32, tag="pm")
                for ci, (o, k) in enumerate(g.KC):
                    nc.tensor.matmul(out=pm[:], lhsT=lt[0:k, ci, :], rhs=Wh_sb[ci][:],
                                     start=(ci == 0), stop=(ci == len(g.KC) - 1))
                msg = msgp.tile([P, H], BF16, tag="msg")
                nc.vector.tensor_tensor(out=msg[:], in0=pm[:], in1=inp_res[:, t, :],
                                        op=AO.add)
                nc.vector.tensor_scalar_max(out=msg[:], in0=msg[:], scalar1=0.0)
                store_msg(it + 1, t, msg)
                if (t + 1) % TCB == 0:
                    ag_m(it + 1, t // TCB)

        # ---- readout pass over atoms (amsg = atom sums of Mf[n_up], local only)
        Z = Mf[n_up]
        for grp in range(g.NG):
            strip = stripp.tile([P, len(g.KC), g.AG], F32, tag="strip")
            fa_tiles = []
            for ci, (o, k) in enumerate(g.AC):
                ft = fbp.tile([k, g.AG], BF16, tag=f"fa{ci}")
                nc.sync.dma_start(out=ft[:], in_=faT[o:o + k, grp * g.AG:(grp + 1) * g.AG])
                fa_tiles.append(ft)
            for ti in range(g.TPG):
                t = grp * g.TPG + ti
                rt = gp.tile([P, 6, H], BF16, tag="rt")
                for kk in range(6):
                    nc.gpsimd.indirect_dma_start(
                        out=rt[:, kk, :], out_offset=None, in_=Z[:, :],
                        in_offset=bass.IndirectOffsetOnAxis(
                            ap=aidx_sb[:, t * 6 + kk:t * 6 + kk + 1], axis=0))
                nc.vector.tensor_tensor(out=rt[:, 0:3, :], in0=rt[:, 0:5:2, :],
                                        in1=rt[:, 1:6:2, :], op=AO.add)
                nc.vector.tensor_tensor(out=rt[:, 0, :], in0=rt[:, 0, :],
                                        in1=rt[:, 1, :], op=AO.add)
                amsg = msgp.tile([P, H], BF16, tag="amsg")
                nc.vector.tensor_tensor(out=amsg[:], in0=rt[:, 0, :],
                                        in1=rt[:, 2, :], op=AO.add)
                # transpose a_msg; then hT = relu(Wo1.T@faT + Wo2.T@amsgT + bo)
                lt = transpose_to_lt(amsg)
                pro = prop.tile([P, len(g.KC), P], F32, tag="pro")
                for mi, (mo, mk) in enumerate(g.KC):
                    for ci, (o, k) in enumerate(g.AC):
                        nc.tensor.matmul(out=pro[0:mk, mi, :],
                                         lhsT=Wo1_sb[ci][:, mo:mo + mk],
                                         rhs=fa_tiles[ci][:, ti * P:(ti + 1) * P],
                                         start=(ci == 0), stop=False)
                    for ci, (o, k) in enumerate(g.KC):
                        nc.tensor.matmul(out=pro[0:mk, mi, :],
                                         lhsT=Wo2_sb[ci][:, mo:mo + mk],
                                         rhs=lt[0:k, ci, :],
                                         start=False, stop=(ci == len(g.KC) - 1))
                    nc.scalar.activation(out=strip[0:mk, mi, ti * P:(ti + 1) * P],
                                         in_=pro[0:mk, mi, :],
                                         func=mybir.ActivationFunctionType.Relu,
                                         bias=bo_sb[0:mk, mi:mi + 1])
            red = redp.tile([P, len(g.KC), g.MPG], F32, tag="red")
            for mi, (mo, mk) in enumerate(g.KC):
                nc.vector.reduce_sum(
                    out=red[0:mk, mi, :],
                    in_=strip[0:mk, mi, :].rearrange("p (m a) -> p m a", a=g.apm),
                    axis=mybir.AxisListType.X)
                nc.vector.tensor_scalar_mul(out=red[0:mk, mi, :], in0=red[0:mk, mi, :],
                                            scalar1=1.0 / g.apm)
                nc.sync.dma_start(out=outT[mi, 0:mk, grp * g.MPG:(grp + 1) * g.MPG],
                                  in_=red[0:mk, mi, :])
    return nc


# ---------------------------------------------------------------- host side
def _prep_core_inputs(g: Geo, c, f_atoms, f_bonds, a2b, b2a, b2revb, W_i, W_h, W_o, b_o):
    H = g.hidden
    bf = ml_dtypes.bfloat16
    b0, b1 = c * g.BS, (c + 1) * g.BS
    a0, a1 = c * g.AS, (c + 1) * g.AS

    fbT = np.zeros((g.bond_fd, g.BSP), dtype=bf)
    fbT[:, :g.BS] = f_bonds[b0:b1].T.astype(bf)
    faT = np.zeros((g.atom_fd, g.ASP), dtype=bf)
    faT[:, :g.AS] = f_atoms[a0:a1].T.astype(bf)

    # global id -> padded chunk-major AllGather table row
    CSB = g.BSP // g.NCHB

    def tabB(j):
        c_, r = j // g.BS, j % g.BS
        return (r // CSB) * (g.n_cores * CSB) + c_ * CSB + (r % CSB)

    CSA = g.ASP // g.NCHA

    def tabA(a):
        c_, r = a // g.AS, a % g.AS
        return (r // CSA) * (g.n_cores * CSA) + c_ * CSA + (r % CSA)

    # atom gather indices: 6x a2b[a] (used for A-builds and readout)
    aa = np.arange(a0, a1)
    ri = np.zeros((g.ASP, 6), dtype=np.int64)
    ri[:g.AS] = a2b[aa]
    ri = tabB(ri)
    aidx = ri.reshape(g.AT, P, 6).transpose(1, 0, 2).reshape(P, g.AT * 6).astype(np.int32)

    # bond gather indices: A[b2a[j]] then M[b2revb[j]]
    jj = np.arange(b0, b1)
    bi = np.zeros((g.BSP, 2), dtype=np.int64)
    bi[:g.BS, 0] = tabA(b2a[jj])
    bi[:g.BS, 1] = tabB(b2revb[jj])
    bidx = bi.reshape(g.BT, P, 2).transpose(1, 0, 2).reshape(P, g.BT * 2).astype(np.int32)

    nk = len(g.KC)
    bo = np.zeros((P, nk), dtype=np.float32)
    for mi, (mo, mk) in enumerate(g.KC):
        bo[:mk, mi] = b_o[mo:mo + mk]

    return {
        "fbT": fbT, "faT": faT, "aidx": aidx, "bidx": bidx,
        "Wi": W_i.astype(bf), "Wh": W_h.astype(bf),
        "Wo1": W_o[:g.atom_fd].astype(bf), "Wo2": W_o[g.atom_fd:].astype(bf),
        "bo": bo,
    }


def _assemble_output(g: Geo, outs):
    H = g.hidden
    n_mols_core = g.mols_per_core
    res = np.empty((g.n_cores * n_mols_core, H), dtype=np.float32)
    for c, om in enumerate(outs):
        o = om["outT"]  # [nk, P, NG*MPG]
        flat = o.reshape(len(g.KC) * P, -1)
        rows = []
        for mi, (mo, mk) in enumerate(g.KC):
            rows.append(flat[mi * P:mi * P + mk])
        hT = np.concatenate(rows, axis=0)  # [H, NG*MPG]
        res[c * n_mols_core:(c + 1) * n_mols_core] = hT[:, :n_mols_core].T
    return res


def kernel(f_atoms, f_bonds, a2b, b2a, b2revb, mol_ids, W_i, W_h, W_o, b_o, _run=None):
    f_atoms = np.asarray(f_atoms, dtype=np.float32)
    f_bonds = np.asarray(f_bonds, dtype=np.float32)
    a2b = np.asarray(a2b, dtype=np.int64)
    b2a = np.asarray(b2a, dtype=np.int64)
    b2revb = np.asarray(b2revb, dtype=np.int64)
    W_i = np.asarray(W_i, dtype=np.float32)
    W_h = np.asarray(W_h, dtype=np.float32)
    W_o = np.asarray(W_o, dtype=np.float32)
    b_o = np.asarray(b_o, dtype=np.float32)

    g = Geo(n_atoms=f_atoms.shape[0], n_bonds=f_bonds.shape[0],
            hidden=W_h.shape[0], atom_fd=f_atoms.shape[1],
            bond_fd=f_bonds.shape[1], n_cores=8, apm=20)

    nc = build(g)
    in_maps = [_prep_core_inputs(g, c, f_atoms, f_bonds, a2b, b2a, b2revb,
                                 W_i, W_h, W_o, b_o) for c in range(g.n_cores)]
    if _run is None:
        split_waits(nc)
        res = run_bass_kernel_spmd(nc, in_maps, core_ids=list(range(g.n_cores)))
        outs = res.results
    else:
        outs = _run(nc, in_maps, g)
    return _assemble_output(g, outs)


# revision 32
# speedup vs baseline: 1.3508x; 1.1051x over previous
"""DMPNN encoder on 8 trn2 NeuronCores (Bass/Tile).

Per-atom formulation (dedups neighbor-sum across bonds sharing a source atom):
    msg_0 = relu(inp),  inp = f_bonds @ W_i            [bond rows, raw bf16]
    (allgather msg_t chunks across cores -> Mf)
    A_t[a]  = sum_k Mf_t[a2b[a,k]]                     [atom rows; 6 gathers/tile]
    (allgather A_t chunks -> Af;  t=0,1 only)
    m_j     = Af_t[b2a[j]] - Mf_t[b2revb[j]]           [2 gathers/bond tile]
    msg_{t+1} = relu(inp + m @ W_h)                    [transpose + matmul on-tile]
    readout: amsg = A_2 (local, no AG); hT = relu(W_o1.T@f_atomsT + (amsg@W_o2)T + b_o)
    out = mean-pool hT over 20-atom molecules (transposed), host re-transposes.

Tables are raw bf16 messages; premultiply by W_h happens after the gather-diff,
which is what lets one A row serve every bond with the same source atom.
AllGathers are split into NCH chunks emitted as soon as their tiles finish, so
only the last chunk sits on the critical path.
"""
import sys
import types
import numpy as np
import ml_dtypes
from contextlib import ExitStack
from dataclasses import dataclass

import concourse.bass as bass
import concourse.mybir as mybir
import concourse.tile as tile
from concourse.masks import make_identity
from concourse.bass_utils import run_bass_kernel_spmd

P = 128
BF16 = mybir.dt.bfloat16
F8 = mybir.dt.float8e4
F32 = mybir.dt.float32
I32 = mybir.dt.int32
AO = mybir.AluOpType


# ---------------------------------------------------------------- wait split
_seq = [0]


def _mk_event(engine, waits, debug):
    _seq[0] += 1
    return mybir.InstEventSemaphore(
        name=f"WS-{_seq[0]}", engine=engine, ins=[], outs=[], debug=debug,
        sync_info=mybir.SyncInfo(on_wait=list(waits), on_update=[]),
    )


def split_waits(nc, cap_normal=1, cap_event=2):
    """This container's walrus caps sync waits at 1/inst (2 for
    EventSemaphore); Tile emits more. Move excess waits onto preceding
    EventSemaphore instructions on the same engine."""
    n_split = 0
    for fn in nc.m.functions:
        for blk in fn.blocks:
            insts = list(blk.instructions)
            out = []
            for inst in insts:
                si = inst.sync_info
                waits = list(si.on_wait) if (si is not None and si.on_wait) else []
                cap = cap_event if isinstance(inst, mybir.InstEventSemaphore) else cap_normal
                if len(waits) > cap:
                    extra, keep = waits[:-cap], waits[-cap:]
                    for i in range(0, len(extra), cap_event):
                        out.append(_mk_event(inst.engine, extra[i:i + cap_event], inst.debug))
                    inst.sync_info = mybir.SyncInfo(on_wait=keep, on_update=list(si.on_update or []))
                    n_split += 1
                out.append(inst)
            if len(out) != len(insts):
                blk.instructions = out
    return n_split


def install_ntff_shim():
    if "antenv.axon_hooks" in sys.modules:
        return
    mod = types.ModuleType("antenv.axon_hooks")
    state = {"hook": None}
    mod.set_axon_ntff_profile_hook = lambda h: state.__setitem__("hook", h)
    mod.get_axon_ntff_profile_hook = lambda: state["hook"]
    sys.modules["antenv.axon_hooks"] = mod
    import antenv
    antenv.axon_hooks = mod
    try:
        from trn_agent_boot.trn_boot import _ntff_profile_via_ctypes
        mod.set_axon_ntff_profile_hook(_ntff_profile_via_ctypes("/opt/axon/libaxon_pjrt.so"))
    except Exception:
        pass


# ---------------------------------------------------------------- geometry
@dataclass
class Geo:
    n_atoms: int
    n_bonds: int
    hidden: int
    atom_fd: int
    bond_fd: int
    n_cores: int
    apm: int  # atoms per mol

    def __post_init__(g):
        assert g.n_bonds % g.n_cores == 0 and g.n_atoms % g.n_cores == 0
        g.BS = g.n_bonds // g.n_cores
        g.AS = g.n_atoms // g.n_cores
        g.BT = -(-g.BS // P)            # bond tiles per core
        g.BSP = g.BT * P
        g.NCHB = 7                      # AllGather chunks (bond tables)
        g.NCHA = 5                      # AllGather chunks (atom tables)
        assert g.BT % g.NCHB == 0
        g.mols_per_core = g.AS // g.apm
        # atom group = lcm(P, apm) atoms so molecule boundaries align
        import math
        g.AG = math.lcm(P, g.apm)       # atoms per pool group
        g.TPG = g.AG // P               # tiles per group
        g.MPG = g.AG // g.apm           # mols per group
        g.NG = -(-g.AS // g.AG)         # groups per core
        g.AT = g.NG * g.TPG             # atom tiles per core (padded)
        g.ASP = g.AT * P
        assert g.AT % g.NCHA == 0
        # hidden chunks of <=128
        g.KC = []
        o = 0
        while o < g.hidden:
            g.KC.append((o, min(P, g.hidden - o)))
            o += P
        # bond_fd chunks
        g.FC = []
        o = 0
        while o < g.bond_fd:
            g.FC.append((o, min(P, g.bond_fd - o)))
            o += P
        # atom_fd chunks
        g.AC = []
        o = 0
        while o < g.atom_fd:
            g.AC.append((o, min(P, g.atom_fd - o)))
            o += P


DEFAULT_GEO = dict(n_atoms=100_000, n_bonds=200_000, hidden=300, atom_fd=133,
                   bond_fd=147, n_cores=8, apm=20)


# ---------------------------------------------------------------- device program
def build(g: Geo, depth: int = 3):
    H = g.hidden
    nc = bass.Bass(num_devices=g.n_cores)

    fbT = nc.declare_dram_parameter("fbT", [g.bond_fd, g.BSP], BF16, isOutput=False)
    faT = nc.declare_dram_parameter("faT", [g.atom_fd, g.ASP], BF16, isOutput=False)
    aidx = nc.declare_dram_parameter("aidx", [P, g.AT * 6], I32, isOutput=False)
    bidx = nc.declare_dram_parameter("bidx", [P, g.BT * 2], I32, isOutput=False)
    Wi = nc.declare_dram_parameter("Wi", [g.bond_fd, H], BF16, isOutput=False)
    Wh = nc.declare_dram_parameter("Wh", [H, H], BF16, isOutput=False)
    Wo1 = nc.declare_dram_parameter("Wo1", [g.atom_fd, H], BF16, isOutput=False)
    Wo2 = nc.declare_dram_parameter("Wo2", [H, H], BF16, isOutput=False)
    bo = nc.declare_dram_parameter("bo", [P, len(g.KC)], F32, isOutput=False)
    outT = nc.declare_dram_parameter("outT", [len(g.KC), P, g.NG * g.MPG], F32, isOutput=True)

    n_up = depth - 1  # message update rounds
    CSB = g.BSP // g.NCHB   # bond rows per AG chunk
    TCB = g.BT // g.NCHB    # bond tiles per chunk
    CSA = g.ASP // g.NCHA   # atom rows per AG chunk
    TCA = g.AT // g.NCHA    # atom tiles per chunk
    # raw message tables: local chunks + allgathered Shared. Intermediate
    # rounds ride in fp8 (halves AllGather traffic, which contends with the
    # gather stream on the DMA engines); the readout-facing last table stays
    # bf16 since its error hits the output directly.
    MDT = [F8] * n_up + [BF16]
    Ml = [[nc.dram_tensor(f"M{t}l{c}", [CSB, H], MDT[t]) for c in range(g.NCHB)]
          for t in range(n_up + 1)]
    Mf = [nc.dram_tensor(f"M{t}f", [g.n_cores * g.BSP, H], MDT[t], addr_space="Shared")
          for t in range(n_up + 1)]
    # per-atom sum tables (t=0..n_up-1 need AG; last is consumed locally)
    Al = [[nc.dram_tensor(f"A{t}l{c}", [CSA, H], F8) for c in range(g.NCHA)]
          for t in range(n_up)]
    Af = [nc.dram_tensor(f"A{t}f", [g.n_cores * g.ASP, H], F8, addr_space="Shared")
          for t in range(n_up)]

    rg = [list(range(g.n_cores))]

    def ag_m(t, ch):
        nc.gpsimd.collective_compute(
            "AllGather", AO.bypass, replica_groups=rg,
            ins=[Ml[t][ch][:, :].opt()],
            outs=[Mf[t][ch * g.n_cores * CSB:(ch + 1) * g.n_cores * CSB, :].opt()])

    def ag_a(t, ch):
        nc.gpsimd.collective_compute(
            "AllGather", AO.bypass, replica_groups=rg,
            ins=[Al[t][ch][:, :].opt()],
            outs=[Af[t][ch * g.n_cores * CSA:(ch + 1) * g.n_cores * CSA, :].opt()])

    with tile.TileContext(nc) as tc, ExitStack() as ctx:
        cst = ctx.enter_context(tc.tile_pool(name="cst", bufs=1))
        inp_p = ctx.enter_context(tc.tile_pool(name="inp", bufs=1))
        fbp = ctx.enter_context(tc.tile_pool(name="fbp", bufs=2))
        gp = ctx.enter_context(tc.tile_pool(name="gp", bufs=4))
        msgp = ctx.enter_context(tc.tile_pool(name="msgp", bufs=3))
        ltp = ctx.enter_context(tc.tile_pool(name="ltp", bufs=3))
        up = ctx.enter_context(tc.tile_pool(name="up", bufs=3))
        stripp = ctx.enter_context(tc.tile_pool(name="stripp", bufs=2))
        redp = ctx.enter_context(tc.tile_pool(name="redp", bufs=2))
        ptrp = ctx.enter_context(tc.tile_pool(name="ptr", bufs=2, space="PSUM"))
        pmmp = ctx.enter_context(tc.tile_pool(name="pmm", bufs=4, space="PSUM"))
        prop = ctx.enter_context(tc.tile_pool(name="pro", bufs=2, space="PSUM"))

        # ---- constants
        ident = cst.tile([P, P], F32)
        make_identity(nc, ident[:])
        identb = cst.tile([P, P], BF16)
        nc.scalar.copy(out=identb[:], in_=ident[:])
        aidx_sb = cst.tile([P, g.AT * 6], I32)
        nc.sync.dma_start(out=aidx_sb[:], in_=aidx[:, :])
        bidx_sb = cst.tile([P, g.BT * 2], I32)
        nc.sync.dma_start(out=bidx_sb[:], in_=bidx[:, :])
        bo_sb = cst.tile([P, len(g.KC)], F32)
        nc.sync.dma_start(out=bo_sb[:], in_=bo[:, :])

        def load_w(param, chunks, name):
            tiles = []
            for (o, k) in chunks:
                t = cst.tile([k, H], BF16, tag=f"{name}{o}")
                nc.sync.dma_start(out=t[:], in_=param[o:o + k, :])
                tiles.append(t)
            return tiles

        Wi_sb = load_w(Wi, g.FC, "wi")
        Wh_sb = load_w(Wh, g.KC, "wh")
        Wo1_sb = load_w(Wo1, g.AC, "wo1")
        Wo2_sb = load_w(Wo2, g.KC, "wo2")

        inp_res = inp_p.tile([P, g.BT, H], BF16)

        def store_msg(t, ti, msg_sb):
            u_tab = Ml[t][ti // TCB]
            nc.sync.dma_start(out=u_tab[(ti % TCB) * P:(ti % TCB) * P + P, :],
                              in_=msg_sb[:])

        # transpose [P, H] -> lt [k-chunks, P] (bf16, via PE + PSUM)
        def transpose_to_lt(src):
            idn = ident if src.dtype == F32 else identb
            ptr = ptrp.tile([P, len(g.KC) * P], src.dtype, tag=f"ptr_{src.dtype}")
            for ci, (o, k) in enumerate(g.KC):
                nc.tensor.transpose(out=ptr[0:k, ci * P:ci * P + P],
                                    in_=src[:, o:o + k], identity=idn[:])
            lt = ltp.tile([P, len(g.KC), P], BF16, tag="lt")
            nfull = sum(1 for (_, k) in g.KC if k == P)
            if nfull:
                nc.scalar.copy(out=lt[:, 0:nfull, :], in_=ptr[:, 0:nfull * P])
            for ci, (o, k) in enumerate(g.KC):
                if k != P:
                    nc.scalar.copy(out=lt[0:k, ci, :], in_=ptr[0:k, ci * P:ci * P + P])
            return lt

        # ---- pass 0: inp = f_bonds@Wi ; msg0 = relu(inp)
        FSTR = 8  # tiles per fbT strip load
        for t0 in range(0, g.BT, FSTR):
            nt = min(FSTR, g.BT - t0)
            fb_tiles = []
            for ci, (o, k) in enumerate(g.FC):
                ft = fbp.tile([k, FSTR * P], BF16, tag=f"fb{ci}")
                nc.sync.dma_start(out=ft[:, 0:nt * P], in_=fbT[o:o + k, t0 * P:(t0 + nt) * P])
                fb_tiles.append(ft)
            for ti in range(nt):
                t = t0 + ti
                pi = pmmp.tile([P, H], F32, tag="pm")
                for ci, (o, k) in enumerate(g.FC):
                    nc.tensor.matmul(out=pi[:], lhsT=fb_tiles[ci][:, ti * P:(ti + 1) * P],
                                     rhs=Wi_sb[ci][:],
                                     start=(ci == 0), stop=(ci == len(g.FC) - 1))
                nc.scalar.copy(out=inp_res[:, t, :], in_=pi[:])
                msg = msgp.tile([P, H], MDT[0], tag="msg8")
                nc.vector.tensor_scalar_max(out=msg[:], in0=pi[:], scalar1=0.0)
                store_msg(0, t, msg)
                if (t + 1) % TCB == 0:
                    ag_m(0, t // TCB)

        # ---- rounds t = 0..n_up-1:
        #   A_t = atom sums of Mf[t]; (AG A_t); msg_{t+1} = relu(inp + (A-rev)@Wh)
        for it in range(n_up):
            # A-build over this core's atoms
            for t in range(g.AT):
                gt = gp.tile([P, 6, H], MDT[it], tag="gt8")
                for kk in range(6):
                    nc.gpsimd.indirect_dma_start(
                        out=gt[:, kk, :], out_offset=None, in_=Mf[it][:, :],
                        in_offset=bass.IndirectOffsetOnAxis(
                            ap=aidx_sb[:, t * 6 + kk:t * 6 + kk + 1], axis=0))
                s1 = msgp.tile([P, 3, H], BF16, tag="s1")
                nc.vector.tensor_tensor(out=s1[:, :, :], in0=gt[:, 0:5:2, :],
                                        in1=gt[:, 1:6:2, :], op=AO.add)
                nc.vector.tensor_tensor(out=s1[:, 0, :], in0=s1[:, 0, :],
                                        in1=s1[:, 1, :], op=AO.add)
                asum = msgp.tile([P, H], F8, tag="asum")
                nc.vector.tensor_tensor(out=asum[:], in0=s1[:, 0, :],
                                        in1=s1[:, 2, :], op=AO.add)
                a_tab = Al[it][t // TCA]
                nc.sync.dma_start(out=a_tab[(t % TCA) * P:(t % TCA) * P + P, :],
                                  in_=asum[:])
                if (t + 1) % TCA == 0:
                    ag_a(it, t // TCA)

            # bond update
            for t in range(g.BT):
                gb = gp.tile([P, 2, H], F8, tag="gb8")
                nc.gpsimd.indirect_dma_start(
                    out=gb[:, 0, :], out_offset=None, in_=Af[it][:, :],
                    in_offset=bass.IndirectOffsetOnAxis(
                        ap=bidx_sb[:, t * 2:t * 2 + 1], axis=0))
                nc.gpsimd.indirect_dma_start(
                    out=gb[:, 1, :], out_offset=None, in_=Mf[it][:, :],
                    in_offset=bass.IndirectOffsetOnAxis(
                        ap=bidx_sb[:, t * 2 + 1:t * 2 + 2], axis=0))
                diff = msgp.tile([P, H], BF16, tag="diff")
                nc.vector.scalar_tensor_tensor(out=diff[:], in0=gb[:, 1, :],
                                               scalar=-1.0, in1=gb[:, 0, :],
                                               op0=AO.mult, op1=AO.add)
                lt = transpose_to_lt(diff)
                pm = pmmp.tile([P, H], F32, tag="pm")
                for ci, (o, k) in enumerate(g.KC):
                    nc.tensor.matmul(out=pm[:], lhsT=lt[0:k, ci, :], rhs=Wh_sb[ci][:],
                                     start=(ci == 0), stop=(ci == len(g.KC) - 1))
                msum = msgp.tile([P, H], BF16, tag="msum")
                nc.vector.tensor_tensor(out=msum[:], in0=pm[:], in1=inp_res[:, t, :],
                                        op=AO.add)
                msg = msgp.tile([P, H], MDT[it + 1], tag=f"msg_{MDT[it + 1]}")
                nc.vector.tensor_scalar_max(out=msg[:], in0=msum[:], scalar1=0.0)
                store_msg(it + 1, t, msg)
                if (t + 1) % TCB == 0:
                    ag_m(it + 1, t // TCB)

        # ---- readout pass over atoms (amsg = atom sums of Mf[n_up], local only)
        Z = Mf[n_up]
        for grp in range(g.NG):
            strip = stripp.tile([P, len(g.KC), g.AG], F32, tag="strip")
            fa_tiles = []
            for ci, (o, k) in enumerate(g.AC):
                ft = fbp.tile([k, g.AG], BF16, tag=f"fa{ci}")
                nc.sync.dma_start(out=ft[:], in_=faT[o:o + k, grp * g.AG:(grp + 1) * g.AG])
                fa_tiles.append(ft)
            for ti in range(g.TPG):
                t = grp * g.TPG + ti
                rt = gp.tile([P, 6, H], BF16, tag="rt")
                for kk in range(6):
                    nc.gpsimd.indirect_dma_start(
                        out=rt[:, kk, :], out_offset=None, in_=Z[:, :],
                        in_offset=bass.IndirectOffsetOnAxis(
                            ap=aidx_sb[:, t * 6 + kk:t * 6 + kk + 1], axis=0))
                nc.vector.tensor_tensor(out=rt[:, 0:3, :], in0=rt[:, 0:5:2, :],
                                        in1=rt[:, 1:6:2, :], op=AO.add)
                nc.vector.tensor_tensor(out=rt[:, 0, :], in0=rt[:, 0, :],
                                        in1=rt[:, 1, :], op=AO.add)
                amsg = msgp.tile([P, H], BF16, tag="amsg")
                nc.vector.tensor_tensor(out=amsg[:], in0=rt[:, 0, :],
                                        in1=rt[:, 2, :], op=AO.add)
                # transpose a_msg; then hT = relu(Wo1.T@faT + Wo2.T@amsgT + bo)
                lt = transpose_to_lt(amsg)
                pro = prop.tile([P, len(g.KC), P], F32, tag="pro")
                for mi, (mo, mk) in enumerate(g.KC):
                    for ci, (o, k) in enumerate(g.AC):
                        nc.tensor.matmul(out=pro[0:mk, mi, :],
                                         lhsT=Wo1_sb[ci][:, mo:mo + mk],
                                         rhs=fa_tiles[ci][:, ti * P:(ti + 1) * P],
                                         start=(ci == 0), stop=False)
                    for ci, (o, k) in enumerate(g.KC):
                        nc.tensor.matmul(out=pro[0:mk, mi, :],
                                         lhsT=Wo2_sb[ci][:, mo:mo + mk],
                                         rhs=lt[0:k, ci, :],
                                         start=False, stop=(ci == len(g.KC) - 1))
                    nc.scalar.activation(out=strip[0:mk, mi, ti * P:(ti + 1) * P],
                                         in_=pro[0:mk, mi, :],
                                         func=mybir.ActivationFunctionType.Relu,
                                         bias=bo_sb[0:mk, mi:mi + 1])
            red = redp.tile([P, len(g.KC), g.MPG], F32, tag="red")
            for mi, (mo, mk) in enumerate(g.KC):
                nc.vector.reduce_sum(
                    out=red[0:mk, mi, :],
                    in_=strip[0:mk, mi, :].rearrange("p (m a) -> p m a", a=g.apm),
                    axis=mybir.AxisListType.X)
                nc.vector.tensor_scalar_mul(out=red[0:mk, mi, :], in0=red[0:mk, mi, :],
                                            scalar1=1.0 / g.apm)
                nc.sync.dma_start(out=outT[mi, 0:mk, grp * g.MPG:(grp + 1) * g.MPG],
                                  in_=red[0:mk, mi, :])
    return nc


# ---------------------------------------------------------------- host side
def _prep_core_inputs(g: Geo, c, f_atoms, f_bonds, a2b, b2a, b2revb, W_i, W_h, W_o, b_o):
    H = g.hidden
    bf = ml_dtypes.bfloat16
    b0, b1 = c * g.BS, (c + 1) * g.BS
    a0, a1 = c * g.AS, (c + 1) * g.AS

    fbT = np.zeros((g.bond_fd, g.BSP), dtype=bf)
    fbT[:, :g.BS] = f_bonds[b0:b1].T.astype(bf)
    faT = np.zeros((g.atom_fd, g.ASP), dtype=bf)
    faT[:, :g.AS] = f_atoms[a0:a1].T.astype(bf)

    # global id -> padded chunk-major AllGather table row
    CSB = g.BSP // g.NCHB

    def tabB(j):
        c_, r = j // g.BS, j % g.BS
        return (r // CSB) * (g.n_cores * CSB) + c_ * CSB + (r % CSB)

    CSA = g.ASP // g.NCHA

    def tabA(a):
        c_, r = a // g.AS, a % g.AS
        return (r // CSA) * (g.n_cores * CSA) + c_ * CSA + (r % CSA)

    # atom gather indices: 6x a2b[a] (used for A-builds and readout)
    aa = np.arange(a0, a1)
    ri = np.zeros((g.ASP, 6), dtype=np.int64)
    ri[:g.AS] = a2b[aa]
    ri = tabB(ri)
    aidx = ri.reshape(g.AT, P, 6).transpose(1, 0, 2).reshape(P, g.AT * 6).astype(np.int32)

    # bond gather indices: A[b2a[j]] then M[b2revb[j]]
    jj = np.arange(b0, b1)
    bi = np.zeros((g.BSP, 2), dtype=np.int64)
    bi[:g.BS, 0] = tabA(b2a[jj])
    bi[:g.BS, 1] = tabB(b2revb[jj])
    bidx = bi.reshape(g.BT, P, 2).transpose(1, 0, 2).reshape(P, g.BT * 2).astype(np.int32)

    nk = len(g.KC)
    bo = np.zeros((P, nk), dtype=np.float32)
    for mi, (mo, mk) in enumerate(g.KC):
        bo[:mk, mi] = b_o[mo:mo + mk]

    return {
        "fbT": fbT, "faT": faT, "aidx": aidx, "bidx": bidx,
        "Wi": W_i.astype(bf), "Wh": W_h.astype(bf),
        "Wo1": W_o[:g.atom_fd].astype(bf), "Wo2": W_o[g.atom_fd:].astype(bf),
        "bo": bo,
    }


def _assemble_output(g: Geo, outs):
    H = g.hidden
    n_mols_core = g.mols_per_core
    res = np.empty((g.n_cores * n_mols_core, H), dtype=np.float32)
    for c, om in enumerate(outs):
        o = om["outT"]  # [nk, P, NG*MPG]
        flat = o.reshape(len(g.KC) * P, -1)
        rows = []
        for mi, (mo, mk) in enumerate(g.KC):
            rows.append(flat[mi * P:mi * P + mk])
        hT = np.concatenate(rows, axis=0)  # [H, NG*MPG]
        res[c * n_mols_core:(c + 1) * n_mols_core] = hT[:, :n_mols_core].T
    return res


def kernel(f_atoms, f_bonds, a2b, b2a, b2revb, mol_ids, W_i, W_h, W_o, b_o, _run=None):
    f_atoms = np.asarray(f_atoms, dtype=np.float32)
    f_bonds = np.asarray(f_bonds, dtype=np.float32)
    a2b = np.asarray(a2b, dtype=np.int64)
    b2a = np.asarray(b2a, dtype=np.int64)
    b2revb = np.asarray(b2revb, dtype=np.int64)
    W_i = np.asarray(W_i, dtype=np.float32)
    W_h = np.asarray(W_h, dtype=np.float32)
    W_o = np.asarray(W_o, dtype=np.float32)
    b_o = np.asarray(b_o, dtype=np.float32)

    g = Geo(n_atoms=f_atoms.shape[0], n_bonds=f_bonds.shape[0],
            hidden=W_h.shape[0], atom_fd=f_atoms.shape[1],
            bond_fd=f_bonds.shape[1], n_cores=8, apm=20)

    nc = build(g)
    in_maps = [_prep_core_inputs(g, c, f_atoms, f_bonds, a2b, b2a, b2revb,
                                 W_i, W_h, W_o, b_o) for c in range(g.n_cores)]
    if _run is None:
        split_waits(nc)
        res = run_bass_kernel_spmd(nc, in_maps, core_ids=list(range(g.n_cores)))
        outs = res.results
    else:
        outs = _run(nc, in_maps, g)
    return _assemble_output(g, outs)
